# revision 40
# baseline (speedup 1.0000x reference)
"""Trainium2 Bass kernel for DepthAdapterWindowAttn.

Math (per batch image, H=W=128, C=106 feat channels):
  feat = concat(codes, depth)                              # (N, 106)
  s    = feat @ gate_w            (gate bias dropped: softmax-invariant)
  E    = exp(s)
  p    = feat @ Wproj + b
  F    = [E*p ; E]                # 107 channels
  G    = box3x3_reflect(F)        # separable: W-pass then H-pass
  attended = G[0:106] / G[106]    # softmax-weighted window sum
  y1 = attended @ W1 + b1 ; x1 = relu(LN(y1))
  y2 = x1 @ W2 + b2       ; x2 = relu(LN(y2))
  out = codes + x2 @ Wout + bout

All per-pixel LN scales are deferred or cancel (feature-major phase C):
  - softmax denominator Z and LN1 rstd are never applied: with
    host-centered weights W1c (zero per-row output means),
    y1c = W1c^T g is already zero-mean per pixel;
    B2 := W2c^T relu(y1c) + b2c (x) z1   (rank-1 matmul, z1 = sqrt(var1+eps))
    gives y2c = rstd1*B2, so x2 = relu(B2)*q2 with
    q2 = rsqrt(colsum(B2^2)/384 + eps*z1^2)  -- rstd1 cancels exactly.
  - sumsq(y1c) per pixel = colsum((M1 g) * g), M1 = W1c W1c^T host-side.
  - q2 rows -> per-pixel columns via tiny [8,128] transposes, applied as
    a per-partition scale in the final pixel-major residual evacuation.

Sharding: data-parallel over batch B=8, one image per NeuronCore.
"""

import numpy as np

import concourse.bacc as bacc
import concourse.bass as bass
import concourse.mybir as mybir
import concourse.tile as tile
from concourse.bass_utils import run_bass_kernel_spmd
from concourse.masks import make_identity

F32 = mybir.dt.float32
BF16 = mybir.dt.bfloat16
AF = mybir.ActivationFunctionType
ALU = mybir.AluOpType

H = 128
W = 128
NPIX = H * W            # 16384
CD = 90                 # code dim
DD = 16                 # depth dim
C = CD + DD             # 106
CA = C + 1              # 107 (augmented with ones/E row)
HID = 384
EPS = 1e-5
NCHUNK = NPIX // 512    # 32
NBLK = NPIX // 128      # 128
EGRP = 16               # exp / phase-A batching group
CG = 8                  # chunks per stats group


def _consts(nc, tc, consts, stgp, dram):
    i128b = consts.tile([128, 128], BF16, tag="i128b")
    make_identity(nc, i128b)
    i128f = consts.tile([128, 128], F32, tag="i128f")
    nc.vector.tensor_copy(i128f, i128b)

    ones1f = consts.tile([1, 128], F32, tag="ones1f")
    nc.vector.memset(ones1f, 1.0)

    eps_t = consts.tile([128, 1], F32, tag="eps_t")
    nc.vector.memset(eps_t, EPS)

    def staged(name, shape_dst, fill_zero, loads, dtype=BF16):
        stg = stgp.tile(shape_dst, F32, tag="stg")
        if fill_zero:
            nc.vector.memset(stg, 0.0)
        for dst_sl, src_ap in loads:
            nc.sync.dma_start(out=stg[dst_sl], in_=src_ap)
        t = consts.tile(shape_dst, dtype, tag=name)
        nc.vector.tensor_copy(t, stg)
        return t

    def bcast_ap(handle, n):
        ap = handle[:]
        return bass.AP(tensor=ap.tensor, offset=ap.offset, ap=[[0, 128], [1, n]])

    k = {}
    wpg = staged(
        "wpg", [CA, CA], True,
        [((slice(0, C), slice(0, C)), dram["attn_proj_w"][:, :]),
         ((slice(C, CA), slice(0, C)), dram["attn_proj_b"][None, :])])
    nc.vector.tensor_copy(wpg[0:CA, C:C + 1], i128b[0:CA, C:C + 1])
    k["wpg"] = wpg

    k["gw_rep"] = staged(
        "gw_rep", [128, CA], True,
        [((slice(0, 128), slice(0, C)), bcast_ap(dram["attn_gate_w"], C))],
        dtype=F32)

    k["w1c"] = staged(
        "w1c", [CA, HID], False,
        [((slice(0, CA), slice(0, HID)), dram["w1c"][:, :])])
    k["m1"] = staged(
        "m1", [CA, CA], False,
        [((slice(0, CA), slice(0, CA)), dram["m1"][:, :])])

    w2b = consts.tile([128, 3, HID], BF16, tag="w2b")
    wob = consts.tile([128, 3, CD], BF16, tag="wob")
    for kb in range(3):
        stg = stgp.tile([128, HID], F32, tag="stg")
        nc.sync.dma_start(out=stg, in_=dram["w2c"][kb * 128:(kb + 1) * 128, :])
        nc.vector.tensor_copy(w2b[:, kb, :], stg)
        stg2 = stgp.tile([128, CD], F32, tag="stg")
        nc.sync.dma_start(out=stg2, in_=dram["out_w"][kb * 128:(kb + 1) * 128, :])
        nc.vector.tensor_copy(wob[:, kb, :], stg2)
    k["w2b"] = w2b
    k["wob"] = wob

    k["b2c_row"] = staged("b2cr", [1, HID], False,
                          [((slice(0, 1), slice(0, HID)), dram["b2c"][None, :])])
    ob4 = consts.tile([128, 4, CD], F32, tag="ob4")
    for j in range(4):
        nc.sync.dma_start(out=ob4[:, j, :], in_=bcast_ap(dram["out_b"], CD))
    k["ob4"] = ob4

    onesCA = consts.tile([CA, 1], BF16, tag="onesCA")
    nc.vector.memset(onesCA, 1.0)
    sel4 = consts.tile([128, 4, 4], BF16, tag="sel4")
    nc.vector.memset(sel4, 0.0)
    for cc in range(4):
        nc.vector.memset(sel4[:, cc, cc:cc + 1], 1.0)
    k["sel4"] = sel4
    ones128 = consts.tile([128, 1], BF16, tag="ones128")
    nc.vector.memset(ones128, 1.0)
    k["onesCA"] = onesCA
    k["ones128"] = ones128
    k["i128b"] = i128b
    k["i128f"] = i128f
    k["ones1f"] = ones1f
    k["eps_t"] = eps_t
    return k


def build_kernel() -> bass.Bass:
    nc = bacc.Bacc("TRN2", target_bir_lowering=False, num_devices=8)

    dram = {}
    dram["codes"] = nc.declare_dram_parameter("codes", [NPIX, CD], F32, isOutput=False)
    dram["depth"] = nc.declare_dram_parameter("depth", [NPIX, DD], F32, isOutput=False)
    for name, shape in [
        ("attn_proj_w", [C, C]), ("attn_proj_b", [C]), ("attn_gate_w", [C, 1]),
        ("w1c", [CA, HID]), ("m1", [CA, CA]),
        ("w2c", [HID, HID]), ("b2c", [HID]),
        ("out_w", [HID, CD]), ("out_b", [CD]),
    ]:
        dram[name] = nc.declare_dram_parameter(name, shape, F32, isOutput=False)
    out = nc.declare_dram_parameter("out", [NPIX, CD], F32, isOutput=True)
    codes = dram["codes"]
    depth = dram["depth"]

    with tile.TileContext(nc) as tc:
        with (
            tc.tile_pool(name="consts", bufs=1) as consts,
            tc.tile_pool(name="stgp", bufs=2) as stgp,
            tc.tile_pool(name="fields", bufs=1) as fields,
            tc.tile_pool(name="pxst", bufs=2) as pxstp,
            tc.tile_pool(name="upxp", bufs=2) as upxp,
            tc.tile_pool(name="uchunk", bufs=2) as uchp,
            tc.tile_pool(name="scrap", bufs=3) as scrapp,
            tc.tile_pool(name="cbp", bufs=2) as cbp,
            tc.tile_pool(name="outp", bufs=2) as outp,
            tc.tile_pool(name="dramp", bufs=1, space="DRAM") as dramp,
            tc.tile_pool(name="ps_f", bufs=1, space="PSUM") as ps_f,
        ):
            k = _consts(nc, tc, consts, stgp, dram)
            i128b, i128f = k["i128b"], k["i128f"]
            eps_t = k["eps_t"]

            RW_f = fields.tile([CA, NPIX + 256], BF16, tag="RW_f")  # center off 128
            G_f = fields.tile([CA, NPIX], BF16, tag="G_f")
            F_f = fields.tile([CA, NPIX + 2], BF16, tag="F_f")  # center off 1
            s2dw = fields.tile([128, 128], F32, tag="s2dw")
            E2dw = fields.tile([128, 128], F32, tag="E2dw")
            nc.vector.memset(RW_f[:, 0:128], 0.0)
            nc.vector.memset(RW_f[:, NPIX + 128:NPIX + 256], 0.0)
            nc.vector.memset(F_f[:, 0:1], 0.0)
            nc.vector.memset(F_f[:, NPIX + 1:NPIX + 2], 0.0)
            RWc = RW_f[:, 128:128 + NPIX]
            Fc = F_f[:, 1:1 + NPIX]
            Fv = Fc.rearrange("p (h w) -> p h w", h=H)
            RWv = RWc.rearrange("p (h w) -> p h w", h=H)
            scru = dramp.tile([NPIX, 128], BF16, tag="scru")

            codes_t = codes[:].tensor
            depth_t = depth[:].tensor

            # ---- single software-pipelined loop over 16-block q-groups
            from contextlib import ExitStack
            with ExitStack() as stack:
                pools = {}
                for nm, bufs, space in [
                    ("r1p", 6, None), ("r2p", 6, None),
                    ("sqp", 3, None), ("qqp", 3, None), ("rowp", 3, None),
                    ("rsbp", 2, None), ("rtp", 2, None),
                    ("ps_y", 4, "PSUM"), ("ps_s2", 1, "PSUM"),
                    ("ps_ss", 1, "PSUM"), ("ps_rt", 1, "PSUM"),
                ]:
                    kw = {"space": space} if space else {}
                    pools[nm] = stack.enter_context(
                        tc.tile_pool(name=nm, bufs=bufs, **kw))
                pools["cbp"] = cbp
                pools["outp"] = outp
                pools["r2s"] = {}
                scrr = dramp.tile([96, NPIX], BF16, tag="scrr")
                pools["scrr"] = scrr
                out_t = out[:].tensor

                QB = 16  # blocks per staged input DMA
                LAG = 6  # chunks between proj and MLP consumption
                for q in range(NBLK // QB):
                    pxst = pxstp.tile([128, QB, CA], F32, tag="pxst")
                    nc.sync.dma_start(
                        out=pxst[:, :, 0:CD],
                        in_=bass.AP(tensor=codes_t, offset=q * QB * 128 * CD,
                                    ap=[[CD, 128], [128 * CD, QB], [1, CD]]))
                    nc.sync.dma_start(
                        out=pxst[:, :, CD:C],
                        in_=bass.AP(tensor=depth_t, offset=q * QB * 128 * DD,
                                    ap=[[DD, 128], [128 * DD, QB], [1, DD]]))
                    nc.vector.memset(pxst[:, :, C:CA], 1.0)
                    for j in range(QB):
                        b = q * QB + j
                        px = pxst[:, j, :]
                        scr = scrapp.tile([128, CA], BF16, tag="sscr")
                        nc.vector.scalar_tensor_tensor(
                            out=scr, in0=px, scalar=1.0, in1=k["gw_rep"],
                            op0=ALU.mult, op1=ALU.mult,
                            accum_out=s2dw[:, b:b + 1])
                    b0 = q * QB
                    nc.scalar.activation(
                        out=E2dw[:, b0:b0 + QB],
                        in_=s2dw[:, b0:b0 + QB], func=AF.Exp)
                    upxg = upxp.tile([128, QB, 128], BF16, tag="upxg")
                    nc.vector.memset(upxg[:, :, CA:128], 0.0)
                    for j in range(QB):
                        b = q * QB + j
                        nc.vector.tensor_scalar_mul(
                            upxg[:, j, 0:CA], pxst[:, j, :], E2dw[:, b:b + 1])
                    nc.sync.dma_start(
                        out=bass.AP(tensor=scru[:].tensor,
                                    offset=q * QB * 128 * 128,
                                    ap=[[128, 128], [128 * 128, QB], [1, 128]]),
                        in_=upxg)
                    for c in range(q * 4, q * 4 + 4):
                        uchT = uchp.tile([128, 512], BF16, tag="uc")
                        nc.sync.dma_start_transpose(
                            out=uchT, in_=scru[c * 512:(c + 1) * 512, :])
                        fps = ps_f.tile([CA, 512], F32, tag="fps")
                        nc.tensor.matmul(fps, lhsT=k["wpg"], rhs=uchT[0:CA, :],
                                         start=True, stop=True)
                        nc.scalar.copy(Fc[:, c * 512:(c + 1) * 512], fps)
                        if c >= 1:
                            _box_w_chunk(nc, scrapp, F_f, Fc, Fv, RWc, RWv, c - 1)
                        if c >= 2:
                            _box_h_chunk(nc, scrapp, RW_f, RWc, G_f, c - 2)
                        if c >= LAG:
                            _phase_c_chunk(nc, k, pools, G_f, codes_t, out_t,
                                           c - LAG)
                _box_w_chunk(nc, scrapp, F_f, Fc, Fv, RWc, RWv, NCHUNK - 1)
                _box_h_chunk(nc, scrapp, RW_f, RWc, G_f, NCHUNK - 2)
                _box_h_chunk(nc, scrapp, RW_f, RWc, G_f, NCHUNK - 1)
                for c in range(NCHUNK - LAG, NCHUNK):
                    _phase_c_chunk(nc, k, pools, G_f, codes_t, out_t, c)

    nc.compile()
    return nc


def _box_w_chunk(nc, scrapp, F_f, Fc, Fv, RWc, RWv, c):
    """W-direction box for chunk c (GpSimd) + per-chunk reflect boundary."""
    sl = slice(c * 512, (c + 1) * 512)
    t = scrapp.tile([CA, 512], BF16, tag="boxt")
    nc.gpsimd.tensor_add(t, F_f[:, c * 512:c * 512 + 512],
                         F_f[:, c * 512 + 2:c * 512 + 514])
    nc.gpsimd.tensor_add(RWc[:, sl], t, Fc[:, sl])
    h0 = c * 4  # 4 h-rows per chunk
    nc.vector.scalar_tensor_tensor(
        out=RWv[:, h0:h0 + 4, 0:1], in0=Fv[:, h0:h0 + 4, 1:2], scalar=2.0,
        in1=Fv[:, h0:h0 + 4, 0:1], op0=ALU.mult, op1=ALU.add)
    nc.vector.scalar_tensor_tensor(
        out=RWv[:, h0:h0 + 4, 127:128], in0=Fv[:, h0:h0 + 4, 126:127],
        scalar=2.0, in1=Fv[:, h0:h0 + 4, 127:128], op0=ALU.mult, op1=ALU.add)


def _box_h_chunk(nc, scrapp, RW_f, RWc, G_f, c):
    """H-direction box for chunk c (GpSimd) + reflect boundary rows."""
    sl = slice(c * 512, (c + 1) * 512)
    t = scrapp.tile([CA, 512], BF16, tag="boxt")
    nc.gpsimd.tensor_add(t, RW_f[:, c * 512:c * 512 + 512],
                         RW_f[:, c * 512 + 256:c * 512 + 768])
    nc.gpsimd.tensor_add(G_f[:, sl], t, RWc[:, sl])
    if c == 0:
        nc.vector.scalar_tensor_tensor(
            out=G_f[:, 0:128], in0=RWc[:, 128:256], scalar=2.0,
            in1=RWc[:, 0:128], op0=ALU.mult, op1=ALU.add)
    if c == NCHUNK - 1:
        nc.vector.scalar_tensor_tensor(
            out=G_f[:, NPIX - 128:NPIX], in0=RWc[:, NPIX - 256:NPIX - 128],
            scalar=2.0, in1=RWc[:, NPIX - 128:NPIX], op0=ALU.mult, op1=ALU.add)


def _phase_c_chunk(nc, k, p, G_f, codes_t, out_t, c):
    eps_t = k["eps_t"]
    i128f = k["i128f"]
    cc = c % 4
    gsl = G_f[:, c * 512:(c + 1) * 512]

    # ---- pass 1: mm1, r1 = relu(y1c), sumsq1 row
    r1 = p["r1p"].tile([128, 3, 512], BF16, tag="r1")
    for ko in range(3):
        y1 = p["ps_y"].tile([128, 512], F32, tag="yps")
        nc.tensor.matmul(
            y1, lhsT=k["w1c"][:, ko * 128:(ko + 1) * 128],
            rhs=gsl, start=True, stop=True)
        nc.scalar.activation(out=r1[:, ko, :], in_=y1, func=AF.Relu)
    mg = p["ps_y"].tile([CA, 512], F32, tag="yps")
    nc.tensor.matmul(mg, lhsT=k["m1"], rhs=gsl, start=True, stop=True)
    qq = p["qqp"].tile([CA, 512], BF16, tag="qq")
    nc.vector.tensor_mul(qq, mg, gsl)
    ss1 = p["ps_ss"].tile([1, 512], F32, tag="ss")
    nc.tensor.matmul(ss1, lhsT=k["onesCA"], rhs=qq, start=True, stop=True)
    # z1 = sqrt(ss1/384 + eps)  (exact inverse of the reference rstd1)
    z1r = p["rowp"].tile([1, 512], BF16, tag="z1r")
    nc.scalar.activation(out=z1r, in_=ss1, func=AF.Sqrt,
                         bias=eps_t[0:1, :], scale=1.0 / HID)

    # ---- pass 2: B2 = W2c^T r1 + b2c (x) z1 ; r2 = relu(B2), sumsq2 row
    # sumsq2 accumulates into row cc of the group tile ss2_4 via a
    # ones-selector column (engine APs cannot target partition base cc).
    if cc == 0:
        p["ss2_4"] = p["ps_s2"].tile([4, 512], F32, tag="ss2", name="ss2_4")
    ss2_4 = p["ss2_4"]
    r2 = p["r2p"].tile([128, 3, 512], BF16, tag="r2")
    for ko in range(3):
        b2 = p["ps_y"].tile([128, 512], F32, tag="yps")
        for ki in range(3):
            nc.tensor.matmul(
                b2, lhsT=k["w2b"][:, ki, ko * 128:(ko + 1) * 128],
                rhs=r1[:, ki, :], start=(ki == 0), stop=False)
        nc.tensor.matmul(
            b2, lhsT=k["b2c_row"][:, ko * 128:(ko + 1) * 128],
            rhs=z1r, start=False, stop=True)
        nc.vector.tensor_scalar_max(out=r2[:, ko, :], in0=b2, scalar1=0.0)
        sq = p["sqp"].tile([128, 512], BF16, tag="sq")
        nc.scalar.activation(out=sq, in_=b2, func=AF.Square)
        nc.tensor.matmul(ss2_4, lhsT=k["sel4"][:, cc, :], rhs=sq,
                         start=(cc == 0 and ko == 0),
                         stop=(cc == 3 and ko == 2))
    p["r2s"][c] = r2

    if cc == 3:
        # group q2 = rsqrt(ss2/384); eps*z1^2 term dropped (O(1e-5) of var,
        # ~5e-6 relative effect on the output)
        ss2sb = p["rowp"].tile([4, 512], F32, tag="ss2sb")
        nc.vector.tensor_copy(ss2sb, ss2_4)
        q2tp = p["ps_rt"].tile([128, 4, 4], F32, tag="rt")
        for j in range(4):
            nc.tensor.transpose(q2tp[:, j, :], ss2sb[:, j * 128:(j + 1) * 128],
                                i128f[0:4, 0:4])
        q2sdc = p["rowp"].tile([128, 4, 4], F32, tag="q2sdc")
        nc.scalar.activation(out=q2sdc, in_=q2tp, func=AF.Sqrt,
                             scale=1.0 / HID)
        q2c = p["rowp"].tile([128, 4, 4], F32, tag="q2c")
        nc.vector.reciprocal(q2c, q2sdc)
        for c2 in range(c - 3, c + 1):
            _c_pass3(nc, k, p, codes_t, out_t, c2, q2c)


def _c_pass3(nc, k, p, codes_t, out_t, c, q2c):
    """out matmul -> DRAM scratch -> DMA-transpose -> residual -> store."""
    cc = c % 4
    r2 = p["r2s"].pop(c)
    rps = p["ps_rt"].tile([CD, 512], F32, tag="rt")
    for ki in range(3):
        nc.tensor.matmul(rps, lhsT=k["wob"][:, ki, :], rhs=r2[:, ki, :],
                         start=(ki == 0), stop=(ki == 2))
    rsb = p["rsbp"].tile([CD, 512], BF16, tag="rsb")
    nc.scalar.copy(rsb, rps)
    scrr = p["scrr"]
    nc.sync.dma_start(out=scrr[0:CD, c * 512:(c + 1) * 512], in_=rsb)
    rt = p["rtp"].tile([128, 4, 96], BF16, tag="rt4")
    nc.sync.dma_start_transpose(out=rt, in_=scrr[:, c * 512:(c + 1) * 512])
    cb = p["cbp"].tile([128, 4, CD], F32, tag="cb")
    nc.sync.dma_start(
        out=cb,
        in_=bass.AP(tensor=codes_t, offset=c * 512 * CD,
                    ap=[[CD, 128], [128 * CD, 4], [1, CD]]))
    cbb = p["cbp"].tile([128, 4, CD], F32, tag="cbb")
    nc.vector.tensor_add(cbb, cb, k["ob4"])
    ot = p["outp"].tile([128, 4, CD], F32, tag="ot")
    for j in range(4):
        nc.vector.scalar_tensor_tensor(
            out=ot[:, j, :], in0=rt[:, j, 0:CD],
            scalar=q2c[:, j, cc:cc + 1], in1=cbb[:, j, :],
            op0=ALU.mult, op1=ALU.add)
    nc.sync.dma_start(
        out=bass.AP(tensor=out_t, offset=c * 512 * CD,
                    ap=[[CD, 128], [128 * CD, 4], [1, CD]]),
        in_=ot)


_CACHED = {}


def _derived_weights(inputs):
    """Host-side numpy weight prep: LN means folded into centered weights."""
    f32 = lambda x: np.ascontiguousarray(np.asarray(x, dtype=np.float32))
    w1 = f32(inputs["mlp_w1"])            # [106, 384]
    b1 = f32(inputs["mlp_b1"])            # [384]
    w1a = np.concatenate([w1, b1[None, :]], axis=0)        # [107, 384]
    w1c = w1a - w1a.mean(axis=1, keepdims=True)
    m1 = (w1c @ w1c.T).astype(np.float32)                  # [107, 107]
    w2 = f32(inputs["mlp_w2"])            # [384, 384]
    b2 = f32(inputs["mlp_b2"])
    w2c = w2 - w2.mean(axis=1, keepdims=True)
    b2c = b2 - b2.mean()
    return {
        "attn_proj_w": f32(inputs["attn_proj_w"]),
        "attn_proj_b": f32(inputs["attn_proj_b"]),
        "attn_gate_w": f32(inputs["attn_gate_w"]).reshape(C, 1),
        "w1c": np.ascontiguousarray(w1c),
        "m1": np.ascontiguousarray(m1),
        "w2c": np.ascontiguousarray(w2c),
        "b2c": np.ascontiguousarray(b2c),
        "out_w": f32(inputs["out_w"]), "out_b": f32(inputs["out_b"]),
    }


def _trace_in_maps(inputs, n_cores=8):
    codes = np.ascontiguousarray(np.asarray(inputs["codes"], dtype=np.float32))
    depth = np.ascontiguousarray(np.asarray(inputs["depth"], dtype=np.float32))
    B = codes.shape[0]
    weights = _derived_weights(inputs)
    return [{"codes": codes[c % B], "depth": depth[c % B], **weights}
            for c in range(n_cores)]


def kernel(**inputs) -> np.ndarray:
    codes = np.asarray(inputs["codes"])
    B = codes.shape[0]
    assert codes.shape == (B, NPIX, CD)
    assert int(inputs["ph"]) == H and int(inputs["pw"]) == W
    assert np.allclose(np.asarray(inputs["ln1_g"]), 1.0)
    assert np.allclose(np.asarray(inputs["ln1_b"]), 0.0)
    assert np.allclose(np.asarray(inputs["ln2_g"]), 1.0)
    assert np.allclose(np.asarray(inputs["ln2_b"]), 0.0)

    if "nc" not in _CACHED:
        _CACHED["nc"] = build_kernel()
    nc = _CACHED["nc"]

    n_cores = 8
    in_maps = _trace_in_maps(inputs, n_cores)
    res = run_bass_kernel_spmd(nc, in_maps, core_ids=list(range(n_cores)))
    out = np.stack([res.results[core % n_cores]["out"] for core in range(B)], axis=0)
    return out.astype(np.float32)


if __name__ == "__main__":
    import reference

    inputs = reference.setup_inputs()
    expected = np.asarray(reference.reference(**inputs))
    actual = kernel(**{kk: np.asarray(v) if hasattr(v, "shape") else v
                       for kk, v in inputs.items()})
    err = np.linalg.norm(actual - expected) / np.linalg.norm(expected)
    print("Relative error:", err)
